# revision 27
# baseline (speedup 1.0000x reference)
"""Trainium2 Bass kernel for nn_EnergyBasedModel (energy-based model free-phase dynamics).

Math (equivalent simplification of the reference):
  Since s stays in [0,1] (clip at each step, s_init in [0,0.1]) and x in [0,1),
  hard_sigmoid(u) == u and hard_sigmoid_deriv(s) == 1 identically. Per step:
    p1 = 0.2*(x @ W0) + 0.2*(s2 @ W1^T)        # drive for first hidden block
    p2 = 0.2*(s1 @ W1) + 0.2*(s3 @ W2^T)
    p3 = 0.2*(s2 @ W2)
    v  = 0.8*s + [p1 p2 p3]
    v1 standardized over the full batch (ddof=1, +1e-8), then clip01 everywhere.

Implementation: pure data parallel over the batch (8 cores x 1024 rows),
feature-major (transposed) layout on device so the batch-norm reduction is a
free-axis reduce and the per-feature scale/bias are per-partition scalars.
Per-step batch statistics go through one 8-core AllReduce of [sum, sumsq]
(8KB), overlapped with the second big matmul.

Precision: matmuls run as float32r (reduced-mantissa fp32 at full PE rate,
fp32 PSUM accumulation).  The state is kept in fp32 "master" tiles (updated
in fp32 by the vector/scalar engines); float32r copies are produced only as
matmul operands, so no rounding error accumulates in the state itself.
C0 = 0.2*(x@W0) is computed once in exact fp32, parked in DRAM, and streamed
back each iteration (SBUF is the limiting resource).  Measured end-to-end
rel-l2 error vs the fp32 reference: ~4e-4 (vs ~5e-3 with rounded state).
"""

import os
import numpy as np

import concourse.bass as bass
import concourse.bacc as bacc
import concourse.mybir as mybir
import concourse.tile as tile
from concourse import bass_utils

F32 = mybir.dt.float32

N_CORES = 8
B = 8192
B_LOC = B // N_CORES          # 1024 batch rows per core
H = 1024                      # hidden width (layers 1 and 2)
HB = H // 128                 # 8 partition-blocks per hidden layer
OUT = 10                      # output layer width
X_DIM = 784                   # input width
XB_FULL = X_DIM // 128        # 6 full 128-blocks
X_TAIL = X_DIM - XB_FULL * 128  # 16
NUM_STATE = 2 * H + OUT       # 2058
NT = B_LOC // 512             # 2 free-dim tiles of 512
NSL = [slice(i * 512, (i + 1) * 512) for i in range(NT)]
MSL = [slice(i * 128, (i + 1) * 128) for i in range(HB)]
N_TOTAL = float(B)            # batch size for stats


def build_module(n_iter: int, mm_mode: str = "float32r", skip_ar: bool = False):
    """Build the SPMD Bass module (one program, runs on all 8 cores).

    skip_ar=True replaces the AllReduce with a local copy (numerically wrong
    across cores) -- used only for TimelineSim profiling / timing probes."""
    R = getattr(mybir.dt, mm_mode)   # matmul operand dtype
    need_round = mm_mode != "float32"

    # dynamic_dma_scratch_size shrunk: no gpsimd/SWDGE dynamic DMAs are used,
    # and the default 16KB/partition carveout is SBUF we need.
    nc = bacc.Bacc("TRN2", target_bir_lowering=False, debug=False,
                   num_devices=N_CORES, dynamic_dma_scratch_size=2048)

    # Per-core DRAM I/O (transposed feature-major shards; weights pre-scaled x0.2)
    sT = nc.dram_tensor("sT", [NUM_STATE, B_LOC], F32, kind="ExternalInput")
    xT = nc.dram_tensor("xT", [X_DIM, B_LOC], F32, kind="ExternalInput")
    w0d = nc.dram_tensor("w0s", [X_DIM, H], F32, kind="ExternalInput")
    w1d = nc.dram_tensor("w1s", [H, H], F32, kind="ExternalInput")
    w1td = nc.dram_tensor("w1ts", [H, H], F32, kind="ExternalInput")
    w2d = nc.dram_tensor("w2s", [H, OUT], F32, kind="ExternalInput")
    w2td = nc.dram_tensor("w2ts", [OUT, H], F32, kind="ExternalInput")
    outT = nc.dram_tensor("outT", [NUM_STATE, B_LOC], F32, kind="ExternalOutput")

    add = mybir.AluOpType.add
    sub = mybir.AluOpType.subtract
    mult = mybir.AluOpType.mult
    amax = mybir.AluOpType.max
    amin = mybir.AluOpType.min
    bypass = mybir.AluOpType.bypass

    with tile.TileContext(nc) as tc:
        with (
            tc.tile_pool(name="persist", bufs=1) as pp,
            tc.tile_pool(name="dram", bufs=2, space="DRAM") as dramp,
        ):
            # ------------- persistent state (fp32 masters) + weights -------------
            s1m = pp.tile([128, HB, B_LOC], F32)    # s1^T  [feat, batch]
            s2m = pp.tile([128, HB, B_LOC], F32)
            s3m = pp.tile([OUT, B_LOC], F32)
            w1 = pp.tile([128, HB, H], R)           # 0.2*W1   (lhsT for s1@W1)
            w1t = pp.tile([128, HB, H], R)          # 0.2*W1^T (lhsT for s2@W1^T)
            w2 = pp.tile([128, HB, OUT], R)         # 0.2*W2   (lhsT for s2@W2)
            w2t = pp.tile([OUT, H], R)              # 0.2*W2^T (lhsT for s3@W2^T)


            # stats / allreduce staging, packed into one tile to avoid
            # per-tile SBUF padding overhead
            small = pp.tile([128, 104], F32)
            accs = small[:, 0:32].rearrange("p (a b c) -> p a b c", b=NT, c=2)
            arin = small[:, 32:48]                  # packed [sum(8) | sumsq(8)]
            arout = small[:, 48:64]
            st_mu = small[:, 64:72]
            st_t = small[:, 72:80]
            st_sig = small[:, 80:88]
            st_a = small[:, 88:96]                  # 1/(sigma+eps)
            st_b = small[:, 96:104]                 # -mu/(sigma+eps)

            c0dram = dramp.tile([128, HB, B_LOC], F32, name="c0dram")

            # ---------------- preamble ----------------
            # s masters: plain fp32 DMA; r-copies via DVE (rounds to float32r)
            for mb in range(HB):
                nc.sync.dma_start(s1m[:, mb, :], sT[mb * 128:(mb + 1) * 128, :])
                nc.sync.dma_start(s2m[:, mb, :],
                                  sT[H + mb * 128:H + (mb + 1) * 128, :])
            nc.sync.dma_start(s3m[:], sT[2 * H:NUM_STATE, :])

            # weights: float32r tiles must be produced by a compute op, so
            # stage fp32 bits through small SBUF tiles + DVE rounding copies.
            # All preamble transients share one pool so its release leaves a
            # single contiguous zone for the loop-phase pools.
            with (
                tc.tile_pool(name="c0psum", bufs=8, space="PSUM") as c0psp,
                tc.tile_pool(name="xw", bufs=2) as xw,
            ):
                def load_w(dst_ap, dram_ap, parts=128):
                    if not need_round:
                        nc.sync.dma_start(dst_ap, dram_ap)
                        return
                    cols = dram_ap.shape[-1]
                    for c0_ in range(0, cols, 512):
                        ce = min(c0_ + 512, cols)
                        stg = xw.tile([128, 512], F32, name="wst", bufs=6)
                        nc.sync.dma_start(stg[0:parts, 0:ce - c0_],
                                          dram_ap[:, c0_:ce])
                        nc.vector.tensor_copy(dst_ap[:, c0_:ce],
                                              stg[0:parts, 0:ce - c0_])

                for kb in range(HB):
                    load_w(w1[:, kb, :], w1d[kb * 128:(kb + 1) * 128, :])
                    load_w(w1t[:, kb, :], w1td[kb * 128:(kb + 1) * 128, :])
                load_w(w2t[:], w2td[:], parts=OUT)
                if need_round:
                    stg = xw.tile([128, HB * OUT], F32, name="w2st", bufs=1)
                    w2sv = stg[:].rearrange("p (kb m) -> p kb m", m=OUT)
                    nc.sync.dma_start(
                        w2sv, w2d[:].rearrange("(kb p) m -> p kb m", p=128))
                    nc.vector.tensor_copy(w2[:], w2sv)
                else:
                    nc.sync.dma_start(
                        w2[:], w2d[:].rearrange("(kb p) m -> p kb m", p=128))

                # C0 = 0.2*(x@W0)^T in exact fp32 (fp32 matmuls), parked in
                # DRAM. x/W0 k-blocks stream through small fp32 tiles
                # (re-read per half).
                NKB = XB_FULL + 1  # 7 k-blocks, last one 16 rows
                for half in range(2):
                    pss = [c0psp.tile([128, 512], F32, name="c0ps")
                           for _ in range(4 * NT)]
                    for kb in range(NKB):
                        kp = slice(0, X_TAIL) if kb == XB_FULL else slice(0, 128)
                        krows = slice(kb * 128, min((kb + 1) * 128, X_DIM))
                        xkb = xw.tile([128, B_LOC], F32, name="xkb")
                        wkb = xw.tile([128, H], F32, name="wkb")
                        nc.sync.dma_start(xkb[kp, :], xT[krows, :])
                        nc.sync.dma_start(wkb[kp, :], w0d[krows, :])
                        for mi in range(4):
                            mb = half * 4 + mi
                            for nt in range(NT):
                                nc.tensor.matmul(
                                    pss[mi * NT + nt][:],
                                    wkb[kp, MSL[mb]], xkb[kp, NSL[nt]],
                                    start=(kb == 0), stop=(kb == NKB - 1))
                    for mi in range(4):
                        mb = half * 4 + mi
                        for nt in range(NT):
                            c0sb = xw.tile([128, 512], F32, name="c0sb")
                            nc.scalar.copy(c0sb[:], pss[mi * NT + nt][:])
                            nc.sync.dma_start(c0dram[:, mb, NSL[nt]], c0sb[:])

            # ---------------- iteration loop (statically unrolled) ----------------
            with (
                tc.tile_pool(name="psum", bufs=4, space="PSUM") as psp,
                tc.tile_pool(name="psum3", bufs=2, space="PSUM") as psp3,
                tc.tile_pool(name="sqpsum", bufs=2, space="PSUM") as sqp,
                tc.tile_pool(name="rcopies", bufs=1) as rp,
                tc.tile_pool(name="c0rot", bufs=4) as c0p,
            ):
                if need_round:
                    s1r = rp.tile([128, HB, B_LOC], R)  # float32r matmul copies
                    s2r = rp.tile([128, HB, B_LOC], R)
                    s3r = rp.tile([OUT, B_LOC], R)
                    nc.vector.tensor_copy(s1r[:], s1m[:])
                    nc.vector.tensor_copy(s2r[:], s2m[:])
                    nc.vector.tensor_copy(s3r[:], s3m[:])
                else:
                    s1r, s2r, s3r = s1m, s2m, s3m
                for it in range(n_iter):
                    # --- P1: psum = 0.2*(s2@W1^T)^T ;
                    #     s1m = 0.8*s1m + c0 + psum  (v, pre-norm, fp32) ---
                    for mb in range(HB):
                        for nt in range(NT):
                            c0t = c0p.tile([128, 512], F32, name="c0t")
                            nc.sync.dma_start(c0t[:], c0dram[:, mb, NSL[nt]])
                            # fold A (can run while matmuls stream):
                            # s1m = 0.8*s1m + c0
                            nc.vector.scalar_tensor_tensor(
                                out=s1m[:, mb, NSL[nt]],
                                in0=s1m[:, mb, NSL[nt]], scalar=0.8, in1=c0t[:],
                                op0=mult, op1=add)
                            ps = psp.tile([128, 512], F32, name="mmps")
                            for kb in range(HB):
                                nc.tensor.matmul(
                                    ps[:], w1t[:, kb, MSL[mb]], s2r[:, kb, NSL[nt]],
                                    start=(kb == 0), stop=(kb == HB - 1))
                            # fold B: s1m += psum, with row-sum accumulation
                            nc.vector.scalar_tensor_tensor(
                                out=s1m[:, mb, NSL[nt]],
                                in0=s1m[:, mb, NSL[nt]], scalar=1.0, in1=ps[:],
                                op0=mult, op1=add,
                                accum_out=accs[:, mb, nt, 0:1])
                            # sum of squares via ScalarE (output to PSUM scratch)
                            sqs = sqp.tile([128, 512], F32, name="sqs")
                            nc.scalar.activation(
                                sqs[:], s1m[:, mb, NSL[nt]],
                                mybir.ActivationFunctionType.Square,
                                accum_out=accs[:, mb, nt, 1:2])

                    # --- stats pack + AllReduce (overlaps P2/P3 matmuls) ---
                    nc.vector.tensor_tensor(arin[:, 0:HB], accs[:, :, 0, 0],
                                            accs[:, :, 1, 0], op=add)
                    nc.vector.tensor_tensor(arin[:, HB:2 * HB], accs[:, :, 0, 1],
                                            accs[:, :, 1, 1], op=add)
                    if skip_ar:
                        nc.vector.tensor_scalar_mul(arout, arin, float(N_CORES))
                    else:
                        cc_in = dramp.tile([128, 2 * HB], F32, name="cc_in")
                        cc_out = dramp.tile([128, 2 * HB], F32, name="cc_out",
                                            addr_space="Shared")
                        nc.sync.dma_start(cc_in[:], arin)
                        nc.gpsimd.collective_compute(
                            "AllReduce", add,
                            replica_groups=[list(range(N_CORES))],
                            ins=[cc_in[:]], outs=[cc_out[:]])
                        nc.sync.dma_start(arout, cc_out[:])
                    # mu = S/n ; var = max(S2 - S*mu, 0)/(n-1); a = 1/(sqrt(var)+eps)
                    S = arout[:, 0:HB]
                    S2 = arout[:, HB:2 * HB]
                    nc.vector.tensor_scalar_mul(st_mu, S, 1.0 / N_TOTAL)
                    nc.vector.tensor_tensor(st_t, S, st_mu, op=mult)
                    nc.vector.tensor_tensor(st_t, S2, st_t, op=sub)
                    nc.vector.tensor_scalar(st_t, st_t, 0.0,
                                            1.0 / (N_TOTAL - 1.0), amax, mult)
                    nc.scalar.sqrt(st_sig, st_t)
                    nc.vector.tensor_scalar_add(st_sig, st_sig, 1e-8)
                    nc.vector.reciprocal(st_a, st_sig)
                    nc.vector.scalar_tensor_tensor(
                        out=st_b, in0=st_mu, scalar=-1.0, in1=st_a,
                        op0=mult, op1=mult)

                    # --- P3: psum3 = 0.2*(s2@W2)^T  (fold deferred past P2) ---
                    ps3 = []
                    for nt in range(NT):
                        p3 = psp3.tile([OUT, 512], F32, name="p3ps")
                        for kb in range(HB):
                            nc.tensor.matmul(
                                p3[:], w2[:, kb, 0:OUT], s2r[:, kb, NSL[nt]],
                                start=(kb == 0), stop=(kb == HB - 1))
                        ps3.append(p3)

                    # --- P2: psum = 0.2*(s1@W1)^T + 0.2*(s3@W2^T)^T ; update s2 ---
                    for mb in range(HB):
                        for nt in range(NT):
                            ps = psp.tile([128, 512], F32, name="mmps")
                            for kb in range(HB):
                                nc.tensor.matmul(
                                    ps[:], w1[:, kb, MSL[mb]], s1r[:, kb, NSL[nt]],
                                    start=(kb == 0), stop=False)
                            nc.tensor.matmul(ps[:], w2t[:, MSL[mb]], s3r[:, NSL[nt]],
                                             start=False, stop=True)
                            # s2m = clip01(0.8*s2m + psum); s2r = round(s2m)
                            nc.vector.scalar_tensor_tensor(
                                out=s2m[:, mb, NSL[nt]],
                                in0=s2m[:, mb, NSL[nt]], scalar=0.8, in1=ps[:],
                                op0=mult, op1=add)
                            nc.vector.tensor_scalar(
                                s2m[:, mb, NSL[nt]], s2m[:, mb, NSL[nt]],
                                0.0, 1.0, amax, amin)
                            if need_round:
                                nc.vector.tensor_copy(s2r[:, mb, NSL[nt]],
                                                      s2m[:, mb, NSL[nt]])

                    # --- s3 update (after P2 consumed old s3r) ---
                    for nt in range(NT):
                        nc.vector.scalar_tensor_tensor(
                            out=s3m[:, NSL[nt]],
                            in0=s3m[:, NSL[nt]], scalar=0.8, in1=ps3[nt][:],
                            op0=mult, op1=add)
                        nc.vector.tensor_scalar(
                            s3m[:, NSL[nt]], s3m[:, NSL[nt]], 0.0, 1.0, amax, amin)
                        if need_round:
                            nc.vector.tensor_copy(s3r[:, NSL[nt]], s3m[:, NSL[nt]])

                    # --- s1m = min(relu(v*a + b), 1); s1r = round(s1m) ---
                    #     (after P2 consumed old s1r)
                    for mb in range(HB):
                        for nt in range(NT):
                            nc.scalar.activation(
                                s1m[:, mb, NSL[nt]], s1m[:, mb, NSL[nt]],
                                mybir.ActivationFunctionType.Relu,
                                bias=st_b[:, mb:mb + 1], scale=st_a[:, mb:mb + 1])
                            if need_round:
                                # min-clamp fused into the float32r copy
                                nc.vector.tensor_scalar_min(
                                    s1r[:, mb, NSL[nt]], s1m[:, mb, NSL[nt]], 1.0)
                            nc.vector.tensor_scalar_min(
                                s1m[:, mb, NSL[nt]], s1m[:, mb, NSL[nt]], 1.0)

                # ---------------- store result (fp32 masters) ----------------
                nc.sync.dma_start(
                    outT[0:H, :].rearrange("(mb p) n -> p mb n", p=128), s1m[:])
                nc.sync.dma_start(
                    outT[H:2 * H, :].rearrange("(mb p) n -> p mb n", p=128), s2m[:])
                nc.sync.dma_start(outT[2 * H:NUM_STATE, :], s3m[:])

    nc.compile()
    return nc


_module_cache: dict = {}


def _get_module(n_iter: int, mm_mode: str):
    key = (n_iter, mm_mode)
    if key not in _module_cache:
        _module_cache[key] = build_module(n_iter, mm_mode)
    return _module_cache[key]


def kernel(s_init, x, W0, W1, W2, n_iter):
    n_iter = int(n_iter)
    mm_mode = os.environ.get("KERNEL_MM_MODE", "float32r")

    s_init = np.ascontiguousarray(np.asarray(s_init, dtype=np.float32))
    x = np.ascontiguousarray(np.asarray(x, dtype=np.float32))
    W0 = np.asarray(W0, dtype=np.float32)
    W1 = np.asarray(W1, dtype=np.float32)
    W2 = np.asarray(W2, dtype=np.float32)

    if n_iter == 0:
        return s_init.copy()

    w0s = np.ascontiguousarray(0.2 * W0)
    w1s = np.ascontiguousarray(0.2 * W1)
    w1ts = np.ascontiguousarray(0.2 * W1.T)
    w2s = np.ascontiguousarray(0.2 * W2)
    w2ts = np.ascontiguousarray(0.2 * W2.T)

    nc = _get_module(n_iter, mm_mode)

    in_maps = []
    for c in range(N_CORES):
        rows = slice(c * B_LOC, (c + 1) * B_LOC)
        in_maps.append({
            "sT": np.ascontiguousarray(s_init[rows].T),
            "xT": np.ascontiguousarray(x[rows].T),
            "w0s": w0s, "w1s": w1s, "w1ts": w1ts, "w2s": w2s, "w2ts": w2ts,
        })

    res = bass_utils.run_bass_kernel_spmd(nc, in_maps,
                                          core_ids=list(range(N_CORES)))
    out = np.empty((B, NUM_STATE), dtype=np.float32)
    for c in range(N_CORES):
        out[c * B_LOC:(c + 1) * B_LOC] = res.results[c]["outT"].T
    return out
